# revision 5
# baseline (speedup 1.0000x reference)
"""BitNetV3 MLP kernel for 8 Trainium2 NeuronCores (v3).

Data-parallel over tokens (8 x 512). Weights are ternarized on the host
(BitNet b1.58 weights are static ternary at inference) and streamed as
fp8_e4m3 {-1,0,+1} in tile-major layout; the PE multiplies bf16
activations against fp8 weights directly (verified exact on HW), so no
on-device weight dequant is needed and weight HBM traffic drops 4x.

Exactness model:
- act quant: q = rint(y * qs) via the scalar engine int16 output cast
  (rint+sat); qs = 127/amax folded with the fwht 1/sqrt(n) norm.
  q in [-128,127] is exact in bf16.
- PE accumulates q (bf16) x ternary (fp8) exactly in fp32 PSUM
  (|sum| <= 8192*127 << 2^24).
- per-token dequant a_c/127 and the global ternary scale s_w fold into
  one per-token post-scale applied on PSUM read by the scalar engine.
- fwht #1 (over H, feeds gate/up) is fp32 on DVE.
- fwht #2 (over I) runs in fp16 (2-byte DVE rate, ~1e-4 per-stage
  rounding): silu(g)*u lands in a per-token 2048-column slab; the 11
  stages at stride <2048 run per slab while the PE works on the next
  gate/up block; only 2 cross-slab stages remain at the end.
- gate/up matmuls are issued token-tile-major so they start as soon as
  the first token tile's fwht/quant is done, hiding most of phase 1.
- down projection runs per token-tile-pair, split in two column halves
  (4 PSUM accumulation groups at a time), so its matmuls overlap the
  fwht/quant of the next pair.
"""

import sys
import numpy as np

if "/opt/trn_rl_repo" not in sys.path:
    sys.path.insert(0, "/opt/trn_rl_repo")

B, S, H, I = 2, 2048, 2048, 8192
N_CORES = 8
T = (B * S) // N_CORES  # tokens per core = 512
TT = T // 128           # token tiles per core = 4
EPS = 1e-5
NORM_H = float(1.0 / np.sqrt(H))
NORM_I = float(1.0 / np.sqrt(I))

KH = H // 128    # 16 k-tiles for gate/up contraction
KI = I // 128    # 64 k-tiles for down contraction
OBLK = 512       # psum out block
SOB = 2048       # gate/up weight DMA super-block (2KB fp8 lines)
NSOB_GU = I // SOB   # 4
NINNER = SOB // OBLK  # 4

_CACHE = {}


def _butterfly(nc, Alu, src, dst, h):
    """One butterfly stage (pairs at stride h) from src AP to dst AP."""
    ca = src.rearrange("p (n two h) -> p n two h", two=2, h=h)
    na = dst.rearrange("p (n two h) -> p n two h", two=2, h=h)
    a = ca[:, :, 0, :]
    b = ca[:, :, 1, :]
    nc.vector.tensor_tensor(na[:, :, 0, :], a, b, Alu.add)
    nc.vector.tensor_tensor(na[:, :, 1, :], a, b, Alu.subtract)


def _build_program():
    import concourse.mybir as mybir
    from concourse import bacc
    from concourse.tile import TileContext
    from concourse.masks import make_identity

    f32 = mybir.dt.float32
    bf16 = mybir.dt.bfloat16
    i16 = mybir.dt.int16
    f16 = mybir.dt.float16
    fp8 = mybir.dt.float8e4
    Alu = mybir.AluOpType
    Act = mybir.ActivationFunctionType
    Axis = mybir.AxisListType

    nc = bacc.Bacc("TRN2", target_bir_lowering=False, debug=False,
                   num_devices=N_CORES)

    x = nc.dram_tensor("x", [T, H], f32, kind="ExternalInput").ap()
    # tile-major fp8 ternary weights:
    # wg8/wu8: row = (sob*KH + k)*128 + r, cols = the 2048 I-columns of sob
    wg8 = nc.dram_tensor("wg8", [NSOB_GU * KH * 128, SOB], fp8,
                         kind="ExternalInput").ap()
    wu8 = nc.dram_tensor("wu8", [NSOB_GU * KH * 128, SOB], fp8,
                         kind="ExternalInput").ap()
    # wd8: row = k*128 + r (k over I), cols = all 2048 of H
    wd8 = nc.dram_tensor("wd8", [KI * 128, H], fp8,
                         kind="ExternalInput").ap()
    # sc: [128, 4] f32: col0=s_g/127, col1=s_u/127, col2=s_d/127
    sc = nc.dram_tensor("sc", [128, 4], f32, kind="ExternalInput").ap()
    out = nc.dram_tensor("out", [T, H], f32, kind="ExternalOutput").ap()

    with TileContext(nc) as tc:
        with (
            tc.tile_pool(name="const", bufs=1) as cpool,
            tc.tile_pool(name="scal", bufs=1) as spool,
            tc.tile_pool(name="hb", bufs=1) as hbpool,
            tc.tile_pool(name="mm", bufs=6, space="PSUM") as ppool,
            tc.tile_pool(name="ptr", bufs=2, space="PSUM") as trpool,
        ):
            ident_bf = cpool.tile([128, 128], bf16)
            make_identity(nc, ident_bf[:])
            sc_sb = cpool.tile([128, 4], f32)
            nc.sync.dma_start(out=sc_sb[:], in_=sc)

            # persistent SBUF state: h (fwht in progress), fp16
            hb = [hbpool.tile([128, I], f16, tag=f"hb{t}", name=f"hb{t}")
                  for t in range(TT)]
            comb_g, comb_u = [], []

            with tc.tile_pool(name="q1T", bufs=1) as q1pool:
                q1T = [q1pool.tile([128, T], bf16, tag=f"q1T_{k}",
                                   name=f"q1T_{k}") for k in range(KH)]

                # ---- phase 1: x -> fwht(H) fp32 -> int16 quant -> q1T ----
                with tc.tile_pool(name="xb", bufs=2) as xpool, \
                     tc.tile_pool(name="q1s", bufs=2) as q1spool:
                    for tt in range(TT):
                        xa = xpool.tile([128, H], f32, tag="xa")
                        xb2 = xpool.tile([128, H], f32, tag="xb2")
                        nc.sync.dma_start(out=xa[:],
                                          in_=x[128 * tt:128 * (tt + 1)])
                        cur, nxt = xa[:], xb2[:]
                        h = 1
                        while h < H:
                            _butterfly(nc, Alu, cur, nxt, h)
                            cur, nxt = nxt, cur
                            h *= 2
                        amax = spool.tile([128, 1], f32, tag=f"amax1_{tt}")
                        nc.vector.tensor_reduce(amax[:], cur, Axis.X,
                                                Alu.max,
                                                apply_absolute_value=True)
                        a_c = spool.tile([128, 1], f32, tag=f"ac1_{tt}")
                        nc.vector.tensor_scalar(a_c[:], amax[:], NORM_H, EPS,
                                                Alu.mult, Alu.max)
                        r1 = spool.tile([128, 1], f32, tag=f"r1_{tt}")
                        nc.vector.reciprocal(r1[:], a_c[:])
                        qs = spool.tile([128, 1], f32, tag=f"qs1_{tt}")
                        nc.vector.tensor_scalar_mul(qs[:], r1[:],
                                                    127.0 * NORM_H)
                        cg = spool.tile([128, 1], f32, tag=f"cg_{tt}")
                        nc.vector.tensor_tensor(cg[:], a_c[:], sc_sb[:, 0:1],
                                                Alu.mult)
                        comb_g.append(cg)
                        cu = spool.tile([128, 1], f32, tag=f"cu_{tt}")
                        nc.vector.tensor_tensor(cu[:], a_c[:], sc_sb[:, 1:2],
                                                Alu.mult)
                        comb_u.append(cu)
                        q_i = q1spool.tile([128, H], i16, tag="q1i")
                        nc.scalar.activation(q_i[:], cur, Act.Copy,
                                             scale=qs[:])
                        q_bf = q1spool.tile([128, H], bf16, tag="q1bf")
                        nc.vector.tensor_copy(q_bf[:], q_i[:])
                        for g in range(KH // 4):
                            ps = trpool.tile([128, 512], bf16, tag="tr")
                            for s4 in range(4):
                                k = 4 * g + s4
                                nc.tensor.transpose(
                                    ps[:, 128 * s4:128 * (s4 + 1)],
                                    q_bf[:, 128 * k:128 * (k + 1)],
                                    ident_bf[:])
                            for s4 in range(4):
                                k = 4 * g + s4
                                nc.scalar.activation(
                                    q1T[k][:, 128 * tt:128 * (tt + 1)],
                                    ps[:, 128 * s4:128 * (s4 + 1)],
                                    Act.Copy)

                # ---- phase 2: gate/up matmuls, h=silu(g)*u, local fwht ----
                with tc.tile_pool(name="wload", bufs=40) as wpool, \
                     tc.tile_pool(name="post", bufs=4) as postpool, \
                     tc.tile_pool(name="slab", bufs=1) as slabpool:
                    scrA = [slabpool.tile([128, SOB], f16, tag=f"scrA{t}",
                                          name=f"scrA{t}")
                            for t in range(TT)]
                    for sob in range(NSOB_GU):
                        gt, ut = [], []
                        for k in range(KH):
                            w = wpool.tile([128, SOB], fp8, tag="w")
                            nc.sync.dma_start(
                                out=w[:],
                                in_=wg8[(sob * KH + k) * 128:
                                        (sob * KH + k) * 128 + 128])
                            gt.append(w)
                        for k in range(KH):
                            w = wpool.tile([128, SOB], fp8, tag="w")
                            nc.sync.dma_start(
                                out=w[:],
                                in_=wu8[(sob * KH + k) * 128:
                                        (sob * KH + k) * 128 + 128])
                            ut.append(w)
                        for inner in range(NINNER):
                            isl = slice(OBLK * inner, OBLK * (inner + 1))
                            ps_g = [ppool.tile([128, OBLK], f32, tag="mm",
                                               name="ps_g")
                                    for _ in range(TT)]
                            for tt in range(TT):
                                for k in range(KH):
                                    nc.tensor.matmul(
                                        ps_g[tt][:],
                                        q1T[k][:, 128 * tt:128 * (tt + 1)],
                                        gt[k][:, isl], start=(k == 0),
                                        stop=(k == KH - 1))
                            g_sbs = []
                            for tt in range(TT):
                                g_sb = postpool.tile([128, OBLK], f16,
                                                     tag=f"gsb{tt}")
                                nc.scalar.activation(g_sb[:], ps_g[tt][:],
                                                     Act.Silu,
                                                     scale=comb_g[tt][:])
                                g_sbs.append(g_sb)
                            ps_u = [ppool.tile([128, OBLK], f32, tag="mm",
                                               name="ps_u")
                                    for _ in range(TT)]
                            for tt in range(TT):
                                for k in range(KH):
                                    nc.tensor.matmul(
                                        ps_u[tt][:],
                                        q1T[k][:, 128 * tt:128 * (tt + 1)],
                                        ut[k][:, isl], start=(k == 0),
                                        stop=(k == KH - 1))
                            for tt in range(TT):
                                u_sb = postpool.tile([128, OBLK], f16,
                                                     tag="usb")
                                nc.scalar.activation(u_sb[:], ps_u[tt][:],
                                                     Act.Copy,
                                                     scale=comb_u[tt][:])
                                nc.vector.tensor_tensor(
                                    scrA[tt][:, isl], g_sbs[tt][:],
                                    u_sb[:], Alu.mult)
                        # 11 fwht stages (h=1..1024) per token tile on the
                        # 2048-col slab; ping-pong scrA <-> hb, ends in hb
                        for tt in range(TT):
                            slab = hb[tt][:, SOB * sob:SOB * (sob + 1)]
                            seq = [scrA[tt][:]]
                            for i in range(10):
                                seq.append(slab if i % 2 == 0
                                           else scrA[tt][:])
                            seq.append(slab)
                            h = 1
                            for i in range(11):
                                _butterfly(nc, Alu, seq[i], seq[i + 1], h)
                                h *= 2

            # ---- phase 3+4: cross fwht, quant, transpose, down matmul ----
            comb_d = [None] * TT
            with tc.tile_pool(name="scr8", bufs=1) as scrpool, \
                 tc.tile_pool(name="q2s", bufs=1) as q2pool, \
                 tc.tile_pool(name="wd", bufs=12) as wdpool, \
                 tc.tile_pool(name="osb", bufs=4) as opool:
                scr = scrpool.tile([128, I], f16)
                q2T = {}
                for tt in range(TT):
                    # 2 cross-slab stages: h = 2048, 4096; hb->scr->hb
                    _butterfly(nc, Alu, hb[tt][:], scr[:], 2048)
                    _butterfly(nc, Alu, scr[:], hb[tt][:], 4096)
                    cur = hb[tt][:]
                    amax = spool.tile([128, 1], f32, tag=f"amax2_{tt}")
                    nc.vector.tensor_reduce(amax[:], cur, Axis.X, Alu.max,
                                            apply_absolute_value=True)
                    a_c = spool.tile([128, 1], f32, tag=f"ac2_{tt}")
                    nc.vector.tensor_scalar(a_c[:], amax[:], NORM_I, EPS,
                                            Alu.mult, Alu.max)
                    r2 = spool.tile([128, 1], f32, tag=f"r2_{tt}")
                    nc.vector.reciprocal(r2[:], a_c[:])
                    qs = spool.tile([128, 1], f32, tag=f"qs2_{tt}")
                    nc.vector.tensor_scalar_mul(qs[:], r2[:],
                                                127.0 * NORM_I)
                    cd = spool.tile([128, 1], f32, tag=f"cd_{tt}")
                    nc.vector.tensor_tensor(cd[:], a_c[:], sc_sb[:, 2:3],
                                            Alu.mult)
                    comb_d[tt] = cd
                    q_i = q2pool.tile([128, I], i16, tag="q2i")
                    nc.scalar.activation(q_i[:], cur, Act.Copy, scale=qs[:])
                    q_bf = q2pool.tile([128, I], bf16, tag="q2bf")
                    nc.vector.tensor_copy(q_bf[:], q_i[:])
                    stage = q2pool.tile([128, KI * 128], bf16,
                                        tag=f"q2T_{tt % 2}",
                                        name=f"q2T_{tt % 2}")
                    for g in range(KI // 4):
                        ps = trpool.tile([128, 512], bf16, tag="tr")
                        for s4 in range(4):
                            k = 4 * g + s4
                            nc.tensor.transpose(
                                ps[:, 128 * s4:128 * (s4 + 1)],
                                q_bf[:, 128 * k:128 * (k + 1)],
                                ident_bf[:])
                        nc.scalar.activation(
                            stage[:, 512 * g:512 * (g + 1)], ps[:],
                            Act.Copy)
                    q2T[tt] = stage

                    if tt % 2 == 1:
                        ta, tb = tt - 1, tt
                        for half in range(2):
                            obs = (2 * half, 2 * half + 1)
                            ps_d = {(t, ob): ppool.tile([128, OBLK], f32,
                                                        tag="mm",
                                                        name="ps_d")
                                    for t in (ta, tb) for ob in obs}
                            for k in range(KI):
                                w = wdpool.tile([128, 1024], fp8, tag="wd")
                                nc.sync.dma_start(
                                    out=w[:],
                                    in_=wd8[128 * k:128 * (k + 1),
                                            1024 * half:1024 * (half + 1)])
                                for t in (ta, tb):
                                    for ob in obs:
                                        osl = slice(
                                            OBLK * (ob - 2 * half),
                                            OBLK * (ob - 2 * half + 1))
                                        nc.tensor.matmul(
                                            ps_d[(t, ob)][:],
                                            q2T[t][:,
                                                   128 * k:128 * (k + 1)],
                                            w[:, osl],
                                            start=(k == 0),
                                            stop=(k == KI - 1))
                            for t in (ta, tb):
                                for ob in obs:
                                    o_sb = opool.tile([128, OBLK], f32,
                                                      tag="out")
                                    nc.scalar.activation(
                                        o_sb[:], ps_d[(t, ob)][:],
                                        Act.Copy, scale=comb_d[t][:])
                                    nc.sync.dma_start(
                                        out=out[128 * t:128 * (t + 1),
                                                OBLK * ob:OBLK * (ob + 1)],
                                        in_=o_sb[:])

    nc.compile()
    return nc


def _get_program():
    if "nc" not in _CACHE:
        _CACHE["nc"] = _build_program()
    return _CACHE["nc"]


def _ternarize(w):
    s = max(float(np.mean(np.abs(w))), EPS)
    q = np.clip(np.round(w / s), -1.0, 1.0).astype(np.float32)
    return q, s


def _make_in_maps(hidden_states, w_gate, w_up, w_down):
    import ml_dtypes

    x2 = np.ascontiguousarray(hidden_states.reshape(B * S, H),
                              dtype=np.float32)
    qg, sg = _ternarize(np.asarray(w_gate, dtype=np.float32))
    qu, su = _ternarize(np.asarray(w_up, dtype=np.float32))
    qd, sd = _ternarize(np.asarray(w_down, dtype=np.float32))

    # gate/up: [I, H] -> wT [H, I] -> tiles [(sob, k), 128, 2048]
    def gu_layout(q):
        wt = q.T  # [H, I]
        t = wt.reshape(KH, 128, NSOB_GU, SOB)        # [k, r, sob, c]
        t = t.transpose(2, 0, 1, 3)                  # [sob, k, r, c]
        return np.ascontiguousarray(
            t.reshape(NSOB_GU * KH * 128, SOB)).astype(
                ml_dtypes.float8_e4m3)

    wg8 = gu_layout(qg)
    wu8 = gu_layout(qu)
    # down: [H, I] -> wT [I, H], row-major (each [128, H] k-tile contiguous)
    wd8 = np.ascontiguousarray(qd.T).astype(ml_dtypes.float8_e4m3)

    sc = np.zeros((128, 4), dtype=np.float32)
    sc[:, 0] = sg / 127.0
    sc[:, 1] = su / 127.0
    sc[:, 2] = sd / 127.0

    in_maps = [
        {
            "x": np.ascontiguousarray(x2[T * j:T * (j + 1)]),
            "wg8": wg8, "wu8": wu8, "wd8": wd8, "sc": sc,
        }
        for j in range(N_CORES)
    ]
    return in_maps


def kernel(hidden_states, w_gate, w_up, w_down, _trace=False):
    from concourse.bass_utils import run_bass_kernel_spmd

    nc = _get_program()
    in_maps = _make_in_maps(hidden_states, w_gate, w_up, w_down)
    res = run_bass_kernel_spmd(nc, in_maps, list(range(N_CORES)),
                               trace=_trace)
    pieces = [res.results[j]["out"] for j in range(N_CORES)]
    out = np.concatenate(pieces, axis=0).reshape(B, S, H)
    out = np.ascontiguousarray(out, dtype=np.float32)
    if _trace:
        return out, res
    return out


# revision 6
# speedup vs baseline: 1.0151x; 1.0151x over previous
"""BitNetV3 MLP kernel for 8 Trainium2 NeuronCores (v3).

Data-parallel over tokens (8 x 512). Weights are ternarized on the host
(BitNet b1.58 weights are static ternary at inference) and streamed as
fp8_e4m3 {-1,0,+1} in tile-major layout; the PE multiplies bf16
activations against fp8 weights directly (verified exact on HW), so no
on-device weight dequant is needed and weight HBM traffic drops 4x.

Exactness model:
- act quant: q = rint(y * qs) via the scalar engine int16 output cast
  (rint+sat); qs = 127/amax folded with the fwht 1/sqrt(n) norm.
  q in [-128,127] is exact in bf16.
- PE accumulates q (bf16) x ternary (fp8) exactly in fp32 PSUM
  (|sum| <= 8192*127 << 2^24).
- per-token dequant a_c/127 and the global ternary scale s_w fold into
  one per-token post-scale applied on PSUM read by the scalar engine.
- fwht #1 (over H, feeds gate/up) is fp32 on DVE.
- fwht #2 (over I) runs in fp16 (2-byte DVE rate, ~1e-4 per-stage
  rounding): silu(g)*u lands in a per-token 2048-column slab; the 11
  stages at stride <2048 run per slab while the PE works on the next
  gate/up block; only 2 cross-slab stages remain at the end.
- gate/up matmuls are issued token-tile-major so they start as soon as
  the first token tile's fwht/quant is done, hiding most of phase 1.
- down projection runs per token-tile-pair, split in two column halves
  (4 PSUM accumulation groups at a time), so its matmuls overlap the
  fwht/quant of the next pair.
"""

import sys
import numpy as np

if "/opt/trn_rl_repo" not in sys.path:
    sys.path.insert(0, "/opt/trn_rl_repo")

B, S, H, I = 2, 2048, 2048, 8192
N_CORES = 8
T = (B * S) // N_CORES  # tokens per core = 512
TT = T // 128           # token tiles per core = 4
EPS = 1e-5
NORM_H = float(1.0 / np.sqrt(H))
NORM_I = float(1.0 / np.sqrt(I))

KH = H // 128    # 16 k-tiles for gate/up contraction
KI = I // 128    # 64 k-tiles for down contraction
OBLK = 512       # psum out block
SOB = 2048       # gate/up weight DMA super-block (2KB fp8 lines)
NSOB_GU = I // SOB   # 4
NINNER = SOB // OBLK  # 4

_CACHE = {}


def _butterfly(nc, Alu, src, dst, h):
    """One butterfly stage (pairs at stride h) from src AP to dst AP."""
    ca = src.rearrange("p (n two h) -> p n two h", two=2, h=h)
    na = dst.rearrange("p (n two h) -> p n two h", two=2, h=h)
    a = ca[:, :, 0, :]
    b = ca[:, :, 1, :]
    nc.vector.tensor_tensor(na[:, :, 0, :], a, b, Alu.add)
    nc.vector.tensor_tensor(na[:, :, 1, :], a, b, Alu.subtract)


def _build_program():
    import concourse.mybir as mybir
    from concourse import bacc
    from concourse.tile import TileContext
    from concourse.masks import make_identity

    f32 = mybir.dt.float32
    bf16 = mybir.dt.bfloat16
    i16 = mybir.dt.int16
    f16 = mybir.dt.float16
    fp8 = mybir.dt.float8e4
    Alu = mybir.AluOpType
    Act = mybir.ActivationFunctionType
    Axis = mybir.AxisListType

    nc = bacc.Bacc("TRN2", target_bir_lowering=False, debug=False,
                   num_devices=N_CORES)

    x = nc.dram_tensor("x", [T, H], f32, kind="ExternalInput").ap()
    # tile-major fp8 ternary weights:
    # wg8/wu8: row = (sob*KH + k)*128 + r, cols = the 2048 I-columns of sob
    wg8 = nc.dram_tensor("wg8", [NSOB_GU * KH * 128, SOB], fp8,
                         kind="ExternalInput").ap()
    wu8 = nc.dram_tensor("wu8", [NSOB_GU * KH * 128, SOB], fp8,
                         kind="ExternalInput").ap()
    # wd8: row = k*128 + r (k over I), cols = all 2048 of H
    wd8 = nc.dram_tensor("wd8", [KI * 128, H], fp8,
                         kind="ExternalInput").ap()
    # sc: [128, 4] f32: col0=s_g/127, col1=s_u/127, col2=s_d/127
    sc = nc.dram_tensor("sc", [128, 4], f32, kind="ExternalInput").ap()
    out = nc.dram_tensor("out", [T, H], f32, kind="ExternalOutput").ap()

    with TileContext(nc) as tc:
        with (
            tc.tile_pool(name="const", bufs=1) as cpool,
            tc.tile_pool(name="scal", bufs=1) as spool,
            tc.tile_pool(name="hb", bufs=1) as hbpool,
            tc.tile_pool(name="mm", bufs=6, space="PSUM") as ppool,
            tc.tile_pool(name="ptr", bufs=2, space="PSUM") as trpool,
        ):
            ident_bf = cpool.tile([128, 128], bf16)
            make_identity(nc, ident_bf[:])
            sc_sb = cpool.tile([128, 4], f32)
            nc.sync.dma_start(out=sc_sb[:], in_=sc)

            # persistent SBUF state: h (fwht in progress), fp16
            hb = [hbpool.tile([128, I], f16, tag=f"hb{t}", name=f"hb{t}")
                  for t in range(TT)]
            comb_g, comb_u = [], []

            with tc.tile_pool(name="q1T", bufs=1) as q1pool:
                q1T = [q1pool.tile([128, T], bf16, tag=f"q1T_{k}",
                                   name=f"q1T_{k}") for k in range(KH)]

                # ---- phase 1: x -> fwht(H) fp32 -> int16 quant -> q1T ----
                with tc.tile_pool(name="xb", bufs=2) as xpool, \
                     tc.tile_pool(name="q1s", bufs=2) as q1spool:
                    for tt in range(TT):
                        xa = xpool.tile([128, H], f32, tag="xa")
                        xb2 = xpool.tile([128, H], f32, tag="xb2")
                        nc.sync.dma_start(out=xa[:],
                                          in_=x[128 * tt:128 * (tt + 1)])
                        cur, nxt = xa[:], xb2[:]
                        h = 1
                        while h < H:
                            _butterfly(nc, Alu, cur, nxt, h)
                            cur, nxt = nxt, cur
                            h *= 2
                        amax = spool.tile([128, 1], f32, tag=f"amax1_{tt}")
                        nc.vector.tensor_reduce(amax[:], cur, Axis.X,
                                                Alu.max,
                                                apply_absolute_value=True)
                        a_c = spool.tile([128, 1], f32, tag=f"ac1_{tt}")
                        nc.vector.tensor_scalar(a_c[:], amax[:], NORM_H, EPS,
                                                Alu.mult, Alu.max)
                        r1 = spool.tile([128, 1], f32, tag=f"r1_{tt}")
                        nc.vector.reciprocal(r1[:], a_c[:])
                        qs = spool.tile([128, 1], f32, tag=f"qs1_{tt}")
                        nc.vector.tensor_scalar_mul(qs[:], r1[:],
                                                    127.0 * NORM_H)
                        cg = spool.tile([128, 1], f32, tag=f"cg_{tt}")
                        nc.vector.tensor_tensor(cg[:], a_c[:], sc_sb[:, 0:1],
                                                Alu.mult)
                        comb_g.append(cg)
                        cu = spool.tile([128, 1], f32, tag=f"cu_{tt}")
                        nc.vector.tensor_tensor(cu[:], a_c[:], sc_sb[:, 1:2],
                                                Alu.mult)
                        comb_u.append(cu)
                        q_i = q1spool.tile([128, H], i16, tag="q1i")
                        nc.scalar.activation(q_i[:], cur, Act.Copy,
                                             scale=qs[:])
                        q_bf = q1spool.tile([128, H], bf16, tag="q1bf")
                        nc.vector.tensor_copy(q_bf[:], q_i[:])
                        for g in range(KH // 4):
                            ps = trpool.tile([128, 512], bf16, tag="tr")
                            for s4 in range(4):
                                k = 4 * g + s4
                                nc.tensor.transpose(
                                    ps[:, 128 * s4:128 * (s4 + 1)],
                                    q_bf[:, 128 * k:128 * (k + 1)],
                                    ident_bf[:])
                            for s4 in range(4):
                                k = 4 * g + s4
                                nc.scalar.activation(
                                    q1T[k][:, 128 * tt:128 * (tt + 1)],
                                    ps[:, 128 * s4:128 * (s4 + 1)],
                                    Act.Copy)

                # ---- phase 2: gate/up matmuls, h=silu(g)*u, local fwht ----
                with tc.tile_pool(name="wload", bufs=40) as wpool, \
                     tc.tile_pool(name="post", bufs=4) as postpool, \
                     tc.tile_pool(name="slab", bufs=1) as slabpool:
                    scrA = [slabpool.tile([128, SOB], f16, tag=f"scrA{t}",
                                          name=f"scrA{t}")
                            for t in range(TT)]
                    for sob in range(NSOB_GU):
                        gt, ut = [], []
                        for k in range(KH):
                            w = wpool.tile([128, SOB], fp8, tag="w")
                            nc.sync.dma_start(
                                out=w[:],
                                in_=wg8[(sob * KH + k) * 128:
                                        (sob * KH + k) * 128 + 128])
                            gt.append(w)
                        for k in range(KH):
                            w = wpool.tile([128, SOB], fp8, tag="w")
                            nc.sync.dma_start(
                                out=w[:],
                                in_=wu8[(sob * KH + k) * 128:
                                        (sob * KH + k) * 128 + 128])
                            ut.append(w)
                        for inner in range(NINNER):
                            isl = slice(OBLK * inner, OBLK * (inner + 1))
                            ps_g = [ppool.tile([128, OBLK], f32, tag="mm",
                                               name="ps_g")
                                    for _ in range(TT)]
                            for tt in range(TT):
                                for k in range(KH):
                                    nc.tensor.matmul(
                                        ps_g[tt][:],
                                        q1T[k][:, 128 * tt:128 * (tt + 1)],
                                        gt[k][:, isl], start=(k == 0),
                                        stop=(k == KH - 1))
                            g_sbs = []
                            for tt in range(TT):
                                g_sb = postpool.tile([128, OBLK], f16,
                                                     tag=f"gsb{tt}")
                                nc.scalar.activation(g_sb[:], ps_g[tt][:],
                                                     Act.Silu,
                                                     scale=comb_g[tt][:])
                                g_sbs.append(g_sb)
                            ps_u = [ppool.tile([128, OBLK], f32, tag="mm",
                                               name="ps_u")
                                    for _ in range(TT)]
                            for tt in range(TT):
                                for k in range(KH):
                                    nc.tensor.matmul(
                                        ps_u[tt][:],
                                        q1T[k][:, 128 * tt:128 * (tt + 1)],
                                        ut[k][:, isl], start=(k == 0),
                                        stop=(k == KH - 1))
                            for tt in range(TT):
                                u_sb = postpool.tile([128, OBLK], f16,
                                                     tag="usb")
                                nc.scalar.activation(u_sb[:], ps_u[tt][:],
                                                     Act.Copy,
                                                     scale=comb_u[tt][:])
                                nc.vector.tensor_tensor(
                                    scrA[tt][:, isl], g_sbs[tt][:],
                                    u_sb[:], Alu.mult)
                                if inner == NINNER - 1:
                                    # 11 fwht stages (h=1..1024) on the
                                    # 2048-col slab, right after this
                                    # tile's last mult so the tail chain
                                    # starts asap; ping-pong scrA <-> hb,
                                    # ends in hb
                                    slab = hb[tt][:,
                                                  SOB * sob:SOB * (sob + 1)]
                                    seq = [scrA[tt][:]]
                                    for i in range(10):
                                        seq.append(slab if i % 2 == 0
                                                   else scrA[tt][:])
                                    seq.append(slab)
                                    h = 1
                                    for i in range(11):
                                        _butterfly(nc, Alu, seq[i],
                                                   seq[i + 1], h)
                                        h *= 2

            # ---- phase 3+4: cross fwht, quant, transpose, down matmul ----
            comb_d = [None] * TT
            with tc.tile_pool(name="scr8", bufs=1) as scrpool, \
                 tc.tile_pool(name="q2s", bufs=1) as q2pool, \
                 tc.tile_pool(name="wd", bufs=12) as wdpool, \
                 tc.tile_pool(name="osb", bufs=4) as opool:
                scr = scrpool.tile([128, I], f16)
                q2T = {}
                for tt in range(TT):
                    # 2 cross-slab stages: h = 2048, 4096; hb->scr->hb
                    _butterfly(nc, Alu, hb[tt][:], scr[:], 2048)
                    _butterfly(nc, Alu, scr[:], hb[tt][:], 4096)
                    cur = hb[tt][:]
                    amax = spool.tile([128, 1], f32, tag=f"amax2_{tt}")
                    nc.vector.tensor_reduce(amax[:], cur, Axis.X, Alu.max,
                                            apply_absolute_value=True)
                    a_c = spool.tile([128, 1], f32, tag=f"ac2_{tt}")
                    nc.vector.tensor_scalar(a_c[:], amax[:], NORM_I, EPS,
                                            Alu.mult, Alu.max)
                    r2 = spool.tile([128, 1], f32, tag=f"r2_{tt}")
                    nc.vector.reciprocal(r2[:], a_c[:])
                    qs = spool.tile([128, 1], f32, tag=f"qs2_{tt}")
                    nc.vector.tensor_scalar_mul(qs[:], r2[:],
                                                127.0 * NORM_I)
                    cd = spool.tile([128, 1], f32, tag=f"cd_{tt}")
                    nc.vector.tensor_tensor(cd[:], a_c[:], sc_sb[:, 2:3],
                                            Alu.mult)
                    comb_d[tt] = cd
                    q_i = q2pool.tile([128, I], i16, tag="q2i")
                    nc.scalar.activation(q_i[:], cur, Act.Copy, scale=qs[:])
                    q_bf = q2pool.tile([128, I], bf16, tag="q2bf")
                    nc.vector.tensor_copy(q_bf[:], q_i[:])
                    stage = q2pool.tile([128, KI * 128], bf16,
                                        tag=f"q2T_{tt % 2}",
                                        name=f"q2T_{tt % 2}")
                    for g in range(KI // 4):
                        ps = trpool.tile([128, 512], bf16, tag="tr")
                        for s4 in range(4):
                            k = 4 * g + s4
                            nc.tensor.transpose(
                                ps[:, 128 * s4:128 * (s4 + 1)],
                                q_bf[:, 128 * k:128 * (k + 1)],
                                ident_bf[:])
                        nc.scalar.activation(
                            stage[:, 512 * g:512 * (g + 1)], ps[:],
                            Act.Copy)
                    q2T[tt] = stage

                    # down projection pass for this token tile alone:
                    # 4 PSUM groups, full-width [128, 2048] fp8 wd tiles
                    ps_d = [ppool.tile([128, OBLK], f32, tag="mm",
                                       name="ps_d") for _ in range(4)]
                    for k in range(KI):
                        w = wdpool.tile([128, H], fp8, tag="wd")
                        nc.sync.dma_start(
                            out=w[:], in_=wd8[128 * k:128 * (k + 1)])
                        for ob in range(4):
                            nc.tensor.matmul(
                                ps_d[ob][:],
                                q2T[tt][:, 128 * k:128 * (k + 1)],
                                w[:, OBLK * ob:OBLK * (ob + 1)],
                                start=(k == 0), stop=(k == KI - 1))
                    for ob in range(4):
                        o_sb = opool.tile([128, OBLK], f32, tag="out")
                        nc.scalar.activation(o_sb[:], ps_d[ob][:],
                                             Act.Copy,
                                             scale=comb_d[tt][:])
                        nc.sync.dma_start(
                            out=out[128 * tt:128 * (tt + 1),
                                    OBLK * ob:OBLK * (ob + 1)],
                            in_=o_sb[:])

    nc.compile()
    return nc


def _get_program():
    if "nc" not in _CACHE:
        _CACHE["nc"] = _build_program()
    return _CACHE["nc"]


def _ternarize(w):
    s = max(float(np.mean(np.abs(w))), EPS)
    q = np.clip(np.round(w / s), -1.0, 1.0).astype(np.float32)
    return q, s


def _make_in_maps(hidden_states, w_gate, w_up, w_down):
    import ml_dtypes

    x2 = np.ascontiguousarray(hidden_states.reshape(B * S, H),
                              dtype=np.float32)
    qg, sg = _ternarize(np.asarray(w_gate, dtype=np.float32))
    qu, su = _ternarize(np.asarray(w_up, dtype=np.float32))
    qd, sd = _ternarize(np.asarray(w_down, dtype=np.float32))

    # gate/up: [I, H] -> wT [H, I] -> tiles [(sob, k), 128, 2048]
    def gu_layout(q):
        wt = q.T  # [H, I]
        t = wt.reshape(KH, 128, NSOB_GU, SOB)        # [k, r, sob, c]
        t = t.transpose(2, 0, 1, 3)                  # [sob, k, r, c]
        return np.ascontiguousarray(
            t.reshape(NSOB_GU * KH * 128, SOB)).astype(
                ml_dtypes.float8_e4m3)

    wg8 = gu_layout(qg)
    wu8 = gu_layout(qu)
    # down: [H, I] -> wT [I, H], row-major (each [128, H] k-tile contiguous)
    wd8 = np.ascontiguousarray(qd.T).astype(ml_dtypes.float8_e4m3)

    sc = np.zeros((128, 4), dtype=np.float32)
    sc[:, 0] = sg / 127.0
    sc[:, 1] = su / 127.0
    sc[:, 2] = sd / 127.0

    in_maps = [
        {
            "x": np.ascontiguousarray(x2[T * j:T * (j + 1)]),
            "wg8": wg8, "wu8": wu8, "wd8": wd8, "sc": sc,
        }
        for j in range(N_CORES)
    ]
    return in_maps


def kernel(hidden_states, w_gate, w_up, w_down, _trace=False):
    from concourse.bass_utils import run_bass_kernel_spmd

    nc = _get_program()
    in_maps = _make_in_maps(hidden_states, w_gate, w_up, w_down)
    res = run_bass_kernel_spmd(nc, in_maps, list(range(N_CORES)),
                               trace=_trace)
    pieces = [res.results[j]["out"] for j in range(N_CORES)]
    out = np.concatenate(pieces, axis=0).reshape(B, S, H)
    out = np.ascontiguousarray(out, dtype=np.float32)
    if _trace:
        return out, res
    return out
